# revision 1
# baseline (speedup 1.0000x reference)
"""Trainium2 Bass kernel for nn_HRMReasoning (8-core data parallel).

Key math: stack_pass is affine (z -> z @ W.T + b composed 6x), so every
segment's L-part (15 stack passes) and H-part (3 stack passes) collapse to
single affine maps; segment t's cumulative map is the t-th power. The ACT
halting trajectory only needs q_t = sigmoid(zh_t @ q_w.T + q_b) where
zh_t = zh_0 @ (P^t).T + d_t, so all 11 segment logits come from ONE matmul
against a folded [256, 22] matrix. The final state is selected by the
halting index m via an indirect-DMA gather from a precomposed power table,
then applied with 2 accumulating matmuls per output tile.

Communication-avoiding halting: instead of an all-reduce per segment (or
even one all-gather), EVERY core evaluates the q partial sums over the
full 4096-row batch (16 matmuls) — all cores run the same arithmetic on
the same replicated activations, so they reach bitwise-identical halting
decisions with zero cross-core communication. On this harness the 8 core
launches are staggered by tens of microseconds, so any collective stalls
every core for the full skew; redundant compute is ~7us and fully local.

Sharding: batch dim block-sharded across 8 cores. The env-id gather /
reset masking / final scatter are data movement done host-side during
shard prep and unshard (general: any ids, dones, truncateds).
"""

import numpy as np

EMBED = 256
NUM_LAYERS = 6
H_CYCLES = 3
L_CYCLES = 5
MMIN = 1
MMAX = 10
T = MMAX + 1          # 11 segments max
B = 4096
N_CORES = 8
BP = B // N_CORES     # 512 rows per core
RT = BP // 128        # 4 row-tiles per core
BLK = 129             # rows per segment block: 128 k-rows + 1 bias row
NCH = B // 512        # 8 n-chunks for the replicated q evaluation

# q logits live on partitions 0:11 (q0) and 32:43 (q1) — partition slices
# must start at multiples of 32 on TRN2.
QW = 64           # q-logit partition width (one-hot padded)
Q1 = 32           # base partition of the q1 block
# constpack column layout ([128, CP_W] f32)
C_GT0 = 0         # [:, 0:64]    padded GT rows 0:128
C_GT1 = 64        # [:, 64:128]  padded GT rows 128:256
C_GROW = 128      # [0:64, 128]  q bias (padded column)
C_MMIN = 129      # [0:11, 129]
C_MLAST = 130     # [0:11, 130]
C_TST = 131       # [0:11, 131:142]
C_TVEC = 142      # [0:11, 142]
C_IOTA = 143      # [:, 143:145]  [p, 128+p]
C_ONESR = 145     # [0, 145:273]  row of 128 ones
C_SEL = 273       # [0:64, 273:284] +-1 q-sum selection (D = sel.T @ ssum)
CP_W = 288


def _compose_stack(W, bvec):
    """Affine map M, c with stack_pass(z) == z @ M.T + c (float64)."""
    M = np.eye(EMBED, dtype=np.float64)
    c = np.zeros(EMBED, dtype=np.float64)
    for i in range(NUM_LAYERS):
        Wi = W[i].astype(np.float64)
        M = Wi @ M
        c = Wi @ c + bvec[i].astype(np.float64)
    return M, c


def _compose_pow(M, c, n):
    Mn = np.eye(EMBED, dtype=np.float64)
    cn = np.zeros(EMBED, dtype=np.float64)
    for _ in range(n):
        cn = M @ cn + c
        Mn = M @ Mn
    return Mn, cn


def _host_consts(L_w, L_b, H_w, H_b, q_w, q_b):
    ML, cL = _compose_stack(L_w, L_b)
    MH, cH = _compose_stack(H_w, H_b)
    MLs, cLs = _compose_pow(ML, cL, 15)   # one segment of L
    MHs, cHs = _compose_pow(MH, cH, 3)    # one segment of H

    q_w64 = q_w.astype(np.float64)
    q_b64 = q_b.astype(np.float64)

    # stack2[t*129 + k, :] = [ML^t.T[k], ML^t.T[k+128], MH^t.T[k], MH^t.T[k+128]]
    # stack2[t*129 + 128, :] = [cL_t, cH_t, ...]
    stack2 = np.zeros((T * BLK, 4 * EMBED), np.float32)
    GT = np.zeros((EMBED, 2 * T), np.float32)
    growT = np.zeros(2 * T, np.float32)

    Mcur = np.eye(EMBED); ccur = np.zeros(EMBED)
    Pcur = np.eye(EMBED); dcur = np.zeros(EMBED)
    for j in range(T):                    # segment t = j+1
        ccur = MLs @ ccur + cLs
        Mcur = MLs @ Mcur
        dcur = MHs @ dcur + cHs
        Pcur = MHs @ Pcur
        base = j * BLK
        MT = Mcur.T.astype(np.float32); PT = Pcur.T.astype(np.float32)
        stack2[base:base + 128, 0:EMBED] = MT[0:128]
        stack2[base:base + 128, EMBED:2 * EMBED] = MT[128:256]
        stack2[base:base + 128, 2 * EMBED:3 * EMBED] = PT[0:128]
        stack2[base:base + 128, 3 * EMBED:] = PT[128:256]
        stack2[base + 128, 0:EMBED] = ccur.astype(np.float32)
        stack2[base + 128, EMBED:2 * EMBED] = dcur.astype(np.float32)
        GT[:, j] = (Pcur.T @ q_w64[0]).astype(np.float32)
        GT[:, T + j] = (Pcur.T @ q_w64[1]).astype(np.float32)
        growT[j] = np.float32(q_w64[0] @ dcur + q_b64[0])
        growT[T + j] = np.float32(q_w64[1] @ dcur + q_b64[1])

    cp = np.zeros((128, CP_W), np.float32)
    cp[:, C_GT0:C_GT0 + T] = GT[0:128, 0:T]
    cp[:, C_GT0 + Q1:C_GT0 + Q1 + T] = GT[0:128, T:2 * T]
    cp[:, C_GT1:C_GT1 + T] = GT[128:256, 0:T]
    cp[:, C_GT1 + Q1:C_GT1 + Q1 + T] = GT[128:256, T:2 * T]
    cp[0:T, C_GROW] = growT[0:T]
    cp[Q1:Q1 + T, C_GROW] = growT[T:2 * T]
    cp[0:T, C_MMIN] = 1.0; cp[0, C_MMIN] = 0.0
    cp[T - 1, C_MLAST] = 1.0
    cp[0:T, C_TST:C_TST + T] = np.triu(np.ones((T, T), np.float32), 1)
    cp[0:T, C_TVEC] = np.arange(T, dtype=np.float32)
    cp[:, C_IOTA] = np.arange(128, dtype=np.float32)
    cp[:, C_IOTA + 1] = np.arange(128, dtype=np.float32) + 128.0
    cp[0, C_ONESR:C_ONESR + 128] = 1.0
    for t in range(T):
        cp[t, C_SEL + t] = 1.0
        cp[Q1 + t, C_SEL + t] = -1.0
    import ml_dtypes
    gtb = np.zeros((128, 2 * QW), np.float32)
    gtb[:, 0:T] = GT[0:128, 0:T]
    gtb[:, Q1:Q1 + T] = GT[0:128, T:2 * T]
    gtb[:, QW:QW + T] = GT[128:256, 0:T]
    gtb[:, QW + Q1:QW + Q1 + T] = GT[128:256, T:2 * T]
    gtb = gtb.astype(ml_dtypes.bfloat16)
    return dict(stack2=stack2, cpk=cp, gtbd=gtb)


def _build_module():
    import concourse.bass as bass
    import concourse.mybir as mybir
    import concourse.tile as tile
    from concourse import bacc
    from contextlib import ExitStack

    f32 = mybir.dt.float32
    bf16 = mybir.dt.bfloat16
    i32 = mybir.dt.int32
    Alu = mybir.AluOpType
    Act = mybir.ActivationFunctionType

    nc = bacc.Bacc("TRN2", target_bir_lowering=False, debug=False,
                   enable_asserts=False, num_devices=N_CORES)

    # I/O.  zfhT: full-batch masked-gathered z_h, transposed [256, 4096]
    #       (replicated to every core for the local halting decision).
    #       zslT/zshT: this core's own 512-column slice of z_l / z_h.
    zfhT = nc.dram_tensor("zfhT", [EMBED, B], bf16, kind="ExternalInput").ap()
    zslT = nc.dram_tensor("zslT", [EMBED, BP], f32, kind="ExternalInput").ap()
    zshT = nc.dram_tensor("zshT", [EMBED, BP], f32, kind="ExternalInput").ap()
    stack2 = nc.dram_tensor("stack2", [T * BLK, 4 * EMBED], f32,
                            kind="ExternalInput").ap()
    cpk = nc.dram_tensor("cpk", [128, CP_W], f32, kind="ExternalInput").ap()
    gtbd = nc.dram_tensor("gtbd", [128, 2 * QW], bf16, kind="ExternalInput").ap()
    zl_out = nc.dram_tensor("zl_out", [BP, EMBED], f32, kind="ExternalOutput").ap()
    zh_out = nc.dram_tensor("zh_out", [BP, EMBED], f32, kind="ExternalOutput").ap()

    with tile.TileContext(nc) as tc, ExitStack() as ctx:
        sb = ctx.enter_context(tc.tile_pool(name="sb", bufs=1))
        ps_q = ctx.enter_context(tc.tile_pool(name="ps_q", bufs=2, space="PSUM"))
        ps_f = ctx.enter_context(tc.tile_pool(name="ps_f", bufs=4, space="PSUM"))
        ps_s = ctx.enter_context(tc.tile_pool(name="ps_s", bufs=1, space="PSUM"))

        # DMA priority: the first q matmul needs qr(0,0), qr(1,0) and gtb;
        # issue those at the head of the two HWDGE queues.
        qrt = {}
        for k in range(2):
            qr = sb.tile([128, 1024], bf16, tag=f"qr{k}0", name=f"qr{k}0")
            (nc.sync if k == 0 else nc.scalar).dma_start(
                qr[:], zfhT[k * 128:(k + 1) * 128, 0:1024])
            qrt[k, 0] = qr
        gtb = sb.tile([128, 2 * QW], bf16, tag="gtb")
        nc.scalar.dma_start(gtb[:], gtbd)
        cp = sb.tile([128, CP_W], f32, tag="cp")
        nc.sync.dma_start(cp[:], cpk)
        onesr = cp[0:1, C_ONESR:C_ONESR + 128]
        warm_ps = ps_s.tile([128, 512], f32, tag="warm")

        # ---- replicated q: logits for all 11 segments over all 4096 rows ----
        # bf16 operands (decision margin |D| ~ 12 vs bf16 sum noise << 1);
        # sigmoid row-sums accumulate during the activation (accum_out);
        # D_t = (sum sig0) - (sum sig1) falls out of one +-1 matmul.
        for g in range(1, 4):
            for k in range(2):
                qr = sb.tile([128, 1024], bf16, tag=f"qr{k}{g}",
                             name=f"qr{k}{g}")
                eng = nc.sync if (2 * g + k) % 2 == 0 else nc.scalar
                eng.dma_start(qr[:], zfhT[k * 128:(k + 1) * 128,
                                          g * 1024:(g + 1) * 1024])
                qrt[k, g] = qr
        ssum8 = sb.tile([QW, NCH], f32, tag="ssum8")
        for c in range(NCH):
            qps = ps_q.tile([QW, 512], f32, tag="qps")
            for k in range(2):
                rhs = qrt[k, c // 2][:, (c % 2) * 512:(c % 2) * 512 + 512]
                nc.tensor.matmul(qps[:], gtb[:, k * QW:(k + 1) * QW], rhs,
                                 start=(k == 0), stop=(k == 1))
            sig = sb.tile([QW, 512], f32, tag="sig", bufs=2)
            nc.scalar.activation(sig[:], qps[:], Act.Sigmoid,
                                 bias=cp[0:QW, C_GROW:C_GROW + 1],
                                 accum_out=ssum8[:, c:c + 1])

        # own-slice activations (stationaries for the final matmuls) — only
        # needed by the finals; loaded behind the q stream.
        zown = {}
        for cname, srct in (("l", zslT), ("h", zshT)):
            for k in range(2):
                zt = sb.tile([128, BP], f32, tag=f"zown_{cname}{k}",
                             name=f"zown_{cname}{k}")
                nc.sync.dma_start(zt[:], srct[k * 128:(k + 1) * 128, :])
                zown[cname, k] = zt
        ssum = sb.tile([QW, 1], f32, tag="ssum")
        nc.vector.reduce_sum(out=ssum[:], in_=ssum8[:],
                             axis=mybir.AxisListType.X)
        Dps = ps_s.tile([T, 1], f32, tag="t")
        nc.tensor.matmul(Dps[:], cp[0:QW, C_SEL:C_SEL + T], ssum[:],
                         start=True, stop=True)

        # ---- halting: first t>=2 with sum0>sum1, else t=11 (one-hot w) ----
        h_sb = sb.tile([T, 1], f32, tag="h1")
        nc.vector.tensor_scalar(out=h_sb[:], in0=Dps[:], scalar1=0.0,
                                scalar2=cp[0:T, C_MMIN:C_MMIN + 1],
                                op0=Alu.is_gt, op1=Alu.mult)
        nc.vector.tensor_tensor(out=h_sb[:], in0=h_sb[:],
                                in1=cp[0:T, C_MLAST:C_MLAST + 1], op=Alu.max)
        cps = ps_s.tile([T, 1], f32, tag="t")
        nc.tensor.matmul(cps[:], cp[0:T, C_TST:C_TST + T], h_sb[:],
                         start=True, stop=True)
        notc = sb.tile([T, 1], f32, tag="notc")
        nc.vector.tensor_scalar(out=notc[:], in0=cps[:], scalar1=-1.0,
                                scalar2=1.0, op0=Alu.mult, op1=Alu.add)
        w_sb = sb.tile([T, 1], f32, tag="wsb")
        nc.vector.tensor_scalar(out=w_sb[:], in0=notc[:], scalar1=0.0,
                                scalar2=h_sb[:], op0=Alu.max, op1=Alu.mult)
        mps = ps_s.tile([1, 1], f32, tag="t")
        nc.tensor.matmul(mps[:], w_sb[:], cp[0:T, C_TVEC:C_TVEC + 1],
                         start=True, stop=True)
        m_sb = sb.tile([1, 1], f32, tag="msb")
        nc.vector.tensor_copy(out=m_sb[:], in_=mps[:])
        bps = ps_s.tile([128, 1], f32, tag="t")
        nc.tensor.matmul(bps[:], onesr, m_sb[:], start=True, stop=True)
        m257 = sb.tile([128, 1], f32, tag="m257")
        nc.vector.tensor_scalar(out=m257[:], in0=bps[:], scalar1=float(BLK),
                                scalar2=None, op0=Alu.mult)
        off_f = sb.tile([128, 1], f32, tag="offf")
        nc.vector.tensor_scalar(out=off_f[:], in0=cp[:, C_IOTA:C_IOTA + 1],
                                scalar1=m257[:], scalar2=None, op0=Alu.add)
        off_i = sb.tile([128, 1], i32, tag="offi")
        nc.vector.tensor_copy(out=off_i[:], in_=off_f[:])
        boff_f = sb.tile([2, 1], f32, tag="bofff")
        nc.vector.tensor_scalar(out=boff_f[:], in0=m257[0:2, :],
                                scalar1=128.0, scalar2=None, op0=Alu.add)
        boff_i = sb.tile([2, 1], i32, tag="boffi")
        nc.vector.tensor_copy(out=boff_i[:], in_=boff_f[:])

        # ---- gather the selected segment's [ML^m.T | MH^m.T] and biases ----
        mselt = sb.tile([128, 4 * EMBED], f32, tag="mselt")
        nc.gpsimd.indirect_dma_start(
            out=mselt[:], out_offset=None, in_=stack2,
            in_offset=bass.IndirectOffsetOnAxis(ap=off_i[:], axis=0))
        msel = {0: mselt[:, 0:2 * EMBED], 1: mselt[:, 2 * EMBED:4 * EMBED]}
        mbias = sb.tile([2, 4 * EMBED], f32, tag="mbias")
        nc.gpsimd.indirect_dma_start(
            out=mbias[:], out_offset=None, in_=stack2,
            in_offset=bass.IndirectOffsetOnAxis(ap=boff_i[:], axis=0))

        # keep the PE busy while the indirect gathers land, so the final
        # matmuls run at the unthrottled clock (idle >3.4us re-throttles).
        # The first (tiny) matmul reads off_f, and the rest chain on the
        # same psum tile, pinning the whole burst into the gather window —
        # otherwise the scheduler hoists it into the q phase.
        nc.tensor.matmul(warm_ps[0:1, 0:1], off_f[:], off_f[:],
                         start=True, stop=True)
        for f in range(8):
            nc.tensor.matmul(warm_ps[0:QW, 0:512], gtb[:, 0:QW],
                             qrt[f % 2, f % 4][:, 0:512],
                             start=True, stop=True)

        # ---- final states: z = z0 @ M_m.T + c_m (row-major out) ----
        # one [128,512] psum group per row-tile: cols 0:256 = zl, 256:512 = zh
        for r in range(RT):
            fps = ps_f.tile([128, 2 * EMBED], f32, tag="fps", name="fps")
            nc.tensor.matmul(fps[:, 0:EMBED],
                             zown["l", 0][:, r * 128:(r + 1) * 128],
                             mselt[:, 0:EMBED], start=True, stop=False)
            nc.tensor.matmul(fps[:, 0:EMBED],
                             zown["l", 1][:, r * 128:(r + 1) * 128],
                             mselt[:, EMBED:2 * EMBED], start=False,
                             stop=False, skip_group_check=True)
            nc.tensor.matmul(fps[:, EMBED:2 * EMBED],
                             zown["h", 0][:, r * 128:(r + 1) * 128],
                             mselt[:, 2 * EMBED:3 * EMBED],
                             start=True, stop=False, skip_group_check=True)
            nc.tensor.matmul(fps[:, EMBED:2 * EMBED],
                             zown["h", 1][:, r * 128:(r + 1) * 128],
                             mselt[:, 3 * EMBED:4 * EMBED],
                             start=False, stop=False, skip_group_check=True)
            nc.tensor.matmul(fps[:], onesr, mbias[0:1, 0:2 * EMBED],
                             start=False, stop=True, skip_group_check=True)
            osb = sb.tile([128, 2 * EMBED], f32, tag="osb", name="osb",
                          bufs=4)
            nc.vector.tensor_copy(out=osb[:], in_=fps[:])
            nc.sync.dma_start(zl_out[r * 128:(r + 1) * 128, :],
                              osb[:, 0:EMBED])
            nc.sync.dma_start(zh_out[r * 128:(r + 1) * 128, :],
                              osb[:, EMBED:2 * EMBED])

    nc.compile()
    return nc


_CACHE = {}


def _get_module():
    if "nc" not in _CACHE:
        _CACHE["nc"] = _build_module()
    return _CACHE["nc"]


TRACE = False
LAST_RESULTS = None


def _prep_inputs(carry_z_l, carry_z_h, ids_full, dones, truncateds, consts):
    """Shard prep: env-id gather + reset mask + feature-major transpose."""
    reset = (dones | truncateds).astype(bool)
    z0l = carry_z_l[ids_full]
    z0h = carry_z_h[ids_full]
    z0l[reset] = 0.0
    z0h[reset] = 0.0
    import ml_dtypes
    zflT = np.ascontiguousarray(z0l.T)
    zfhT = np.ascontiguousarray(z0h.T)
    zfhT_bf = np.ascontiguousarray(zfhT.astype(ml_dtypes.bfloat16))
    in_maps = []
    for c in range(N_CORES):
        m = dict(consts)
        m["zfhT"] = zfhT_bf
        m["zslT"] = np.ascontiguousarray(zflT[:, c * BP:(c + 1) * BP])
        m["zshT"] = np.ascontiguousarray(zfhT[:, c * BP:(c + 1) * BP])
        in_maps.append(m)
    return in_maps


def kernel(x, carry_z_l, carry_z_h, L_w, L_b, H_w, H_b, q_w, q_b,
           training_env_ids, dones, truncateds):
    global LAST_RESULTS
    from concourse.bass_utils import run_bass_kernel_spmd

    carry_z_l = np.ascontiguousarray(np.asarray(carry_z_l, np.float32))
    carry_z_h = np.ascontiguousarray(np.asarray(carry_z_h, np.float32))
    ids_full = np.asarray(training_env_ids, np.int32)
    dones = np.asarray(dones).astype(bool)
    truncateds = np.asarray(truncateds).astype(bool)

    consts = _host_consts(np.asarray(L_w, np.float32), np.asarray(L_b, np.float32),
                          np.asarray(H_w, np.float32), np.asarray(H_b, np.float32),
                          np.asarray(q_w, np.float32), np.asarray(q_b, np.float32))
    in_maps = _prep_inputs(carry_z_l, carry_z_h, ids_full, dones,
                           truncateds, consts)

    nc = _get_module()
    res = run_bass_kernel_spmd(nc, in_maps, core_ids=list(range(N_CORES)),
                               trace=TRACE)
    LAST_RESULTS = res

    zl_full = np.concatenate([res.results[c]["zl_out"] for c in range(N_CORES)], 0)
    zh_full = np.concatenate([res.results[c]["zh_out"] for c in range(N_CORES)], 0)

    new_czl = carry_z_l.copy()
    new_czh = carry_z_h.copy()
    new_czl[ids_full] = zl_full
    new_czh[ids_full] = zh_full
    return zh_full, new_czl, new_czh



# revision 2
# speedup vs baseline: 1.0872x; 1.0872x over previous
"""Trainium2 Bass kernel for nn_HRMReasoning (8-core data parallel).

Key math: stack_pass is affine (z -> z @ W.T + b composed 6x), so every
segment's L-part (15 stack passes) and H-part (3 stack passes) collapse to
single affine maps; segment t's cumulative map is the t-th power. The ACT
halting trajectory only needs q_t = sigmoid(zh_t @ q_w.T + q_b) where
zh_t = zh_0 @ (P^t).T + d_t, so all 11 segment logits come from ONE matmul
against a folded [256, 22] matrix. The final state is selected by the
halting index m via an indirect-DMA gather from a precomposed power table,
then applied with 2 accumulating matmuls per output tile.

Communication-avoiding halting: instead of an all-reduce per segment,
EVERY core evaluates the q partial sums over the full 4096-row batch —
all cores run the same arithmetic on the same replicated activations, so
they reach bitwise-identical halting decisions with zero cross-core
communication (core launches are staggered; collectives would stall).

Perf structure (v2):
- dependency-free warm-up matmuls issue from the start of the kernel so
  the PE HAM clock-gate reaches 2.4 GHz before the first real matmul.
- q stream: 8x [128,1024] bf16 DMA chunks; 4 sigmoids of [64,1024] with
  the +-1 halting-sum matmul folded per-chunk (off the critical tail).
- halting chain compressed: one-hot w -> gather offsets via a single
  matmul against a precomputed [t,p]->128t+p table.
- finals in transposed orientation (out = M @ z0T), bf16 operands at
  N=512, bias added during the PSUM evacuation (split ACT/DVE), f32
  biases via a tiny second indirect gather.
"""

import numpy as np

EMBED = 256
NUM_LAYERS = 6
H_CYCLES = 3
L_CYCLES = 5
MMIN = 1
MMAX = 10
T = MMAX + 1          # 11 segments max
B = 4096
N_CORES = 8
BP = B // N_CORES     # 512 rows per core
NCH = 4               # q evaluated in 4 chunks of 1024 rows

# q logits live on partitions 0:11 (q0) and 32:43 (q1) — partition slices
# must start at multiples of 32 on TRN2.
QW = 64           # q-logit partition width (one-hot padded)
Q1 = 32           # base partition of the q1 block

# constpack column layout ([128, CP_W] f32)
C_GROW = 0        # [0:64, 0]      q bias column
C_MMIN = 1        # [0:11, 1]
C_MLAST = 2       # [0:11, 2]
C_TST = 3         # [0:11, 3:14]   strict upper-tri ones
C_SEL = 14        # [0:64, 14:25]  +-1 q-sum selection
C_OFFT = 32       # [0:11, 32:160] offset table: 128*t + p
CP_W = 160


def _compose_stack(W, bvec):
    """Affine map M, c with stack_pass(z) == z @ M.T + c (float64)."""
    M = np.eye(EMBED, dtype=np.float64)
    c = np.zeros(EMBED, dtype=np.float64)
    for i in range(NUM_LAYERS):
        Wi = W[i].astype(np.float64)
        M = Wi @ M
        c = Wi @ c + bvec[i].astype(np.float64)
    return M, c


def _compose_pow(M, c, n):
    Mn = np.eye(EMBED, dtype=np.float64)
    cn = np.zeros(EMBED, dtype=np.float64)
    for _ in range(n):
        cn = M @ cn + c
        Mn = M @ Mn
    return Mn, cn


def _host_consts(L_w, L_b, H_w, H_b, q_w, q_b):
    import ml_dtypes
    ML, cL = _compose_stack(L_w, L_b)
    MH, cH = _compose_stack(H_w, H_b)
    MLs, cLs = _compose_pow(ML, cL, 15)   # one segment of L
    MHs, cHs = _compose_pow(MH, cH, 3)    # one segment of H

    q_w64 = q_w.astype(np.float64)
    q_b64 = q_b.astype(np.float64)

    # stack2b[t*128 + k, :] = [ML^t.T[k] | ML^t.T[k+128] | MH^t.T[k] | MH^t.T[k+128]]
    stack2b = np.zeros((T * 128, 4 * EMBED), np.float32)
    # biasT[t*128 + p, 0:4] = [cL_t[p], cL_t[128+p], cH_t[p], cH_t[128+p]]
    biasT = np.zeros((T * 128, 128), np.float32)
    GT = np.zeros((EMBED, 2 * T), np.float32)
    growT = np.zeros(2 * T, np.float32)

    Mcur = np.eye(EMBED); ccur = np.zeros(EMBED)
    Pcur = np.eye(EMBED); dcur = np.zeros(EMBED)
    for j in range(T):                    # segment t = j+1
        ccur = MLs @ ccur + cLs
        Mcur = MLs @ Mcur
        dcur = MHs @ dcur + cHs
        Pcur = MHs @ Pcur
        base = j * 128
        MT = Mcur.T.astype(np.float32); PT = Pcur.T.astype(np.float32)
        stack2b[base:base + 128, 0:EMBED] = MT[0:128]
        stack2b[base:base + 128, EMBED:2 * EMBED] = MT[128:256]
        stack2b[base:base + 128, 2 * EMBED:3 * EMBED] = PT[0:128]
        stack2b[base:base + 128, 3 * EMBED:] = PT[128:256]
        biasT[base:base + 128, 0] = ccur[0:128].astype(np.float32)
        biasT[base:base + 128, 1] = ccur[128:256].astype(np.float32)
        biasT[base:base + 128, 2] = dcur[0:128].astype(np.float32)
        biasT[base:base + 128, 3] = dcur[128:256].astype(np.float32)
        GT[:, j] = (Pcur.T @ q_w64[0]).astype(np.float32)
        GT[:, T + j] = (Pcur.T @ q_w64[1]).astype(np.float32)
        growT[j] = np.float32(q_w64[0] @ dcur + q_b64[0])
        growT[T + j] = np.float32(q_w64[1] @ dcur + q_b64[1])

    cp = np.zeros((128, CP_W), np.float32)
    cp[0:T, C_GROW] = growT[0:T]
    cp[Q1:Q1 + T, C_GROW] = growT[T:2 * T]
    cp[0:T, C_MMIN] = 1.0; cp[0, C_MMIN] = 0.0
    cp[T - 1, C_MLAST] = 1.0
    cp[0:T, C_TST:C_TST + T] = np.triu(np.ones((T, T), np.float32), 1)
    for t in range(T):
        cp[t, C_SEL + t] = 1.0
        cp[Q1 + t, C_SEL + t] = -1.0
        cp[t, C_OFFT:C_OFFT + 128] = 128.0 * t + np.arange(128, dtype=np.float32)

    gtb = np.zeros((128, 2 * QW), np.float32)
    gtb[:, 0:T] = GT[0:128, 0:T]
    gtb[:, Q1:Q1 + T] = GT[0:128, T:2 * T]
    gtb[:, QW:QW + T] = GT[128:256, 0:T]
    gtb[:, QW + Q1:QW + Q1 + T] = GT[128:256, T:2 * T]
    gtb = gtb.astype(ml_dtypes.bfloat16)
    stack2b = stack2b.astype(ml_dtypes.bfloat16)
    return dict(stack2b=stack2b, biasT=biasT, cpk=cp, gtbd=gtb)


def _build_module():
    import concourse.bass as bass
    import concourse.mybir as mybir
    import concourse.tile as tile
    from concourse import bacc
    from contextlib import ExitStack

    f32 = mybir.dt.float32
    bf16 = mybir.dt.bfloat16
    i32 = mybir.dt.int32
    Alu = mybir.AluOpType
    Act = mybir.ActivationFunctionType

    nc = bacc.Bacc("TRN2", target_bir_lowering=False, debug=False,
                   enable_asserts=False, num_devices=N_CORES)

    # I/O.  zfhT: full-batch masked-gathered z_h, transposed [256, 4096]
    #       (replicated to every core for the local halting decision).
    #       zslT/zshT: this core's own 512-column slice of z_l / z_h.
    zfhT = nc.dram_tensor("zfhT", [EMBED, B], bf16, kind="ExternalInput").ap()
    zslT = nc.dram_tensor("zslT", [EMBED, BP], bf16, kind="ExternalInput").ap()
    zshT = nc.dram_tensor("zshT", [EMBED, BP], bf16, kind="ExternalInput").ap()
    stack2b = nc.dram_tensor("stack2b", [T * 128, 4 * EMBED], bf16,
                             kind="ExternalInput").ap()
    biasT = nc.dram_tensor("biasT", [T * 128, 128], f32,
                           kind="ExternalInput").ap()
    cpk = nc.dram_tensor("cpk", [128, CP_W], f32, kind="ExternalInput").ap()
    gtbd = nc.dram_tensor("gtbd", [128, 2 * QW], bf16, kind="ExternalInput").ap()
    zloT = nc.dram_tensor("zloT", [EMBED, BP], f32, kind="ExternalOutput").ap()
    zhoT = nc.dram_tensor("zhoT", [EMBED, BP], f32, kind="ExternalOutput").ap()

    with tile.TileContext(nc) as tc, ExitStack() as ctx:
        sb = ctx.enter_context(tc.tile_pool(name="sb", bufs=1))
        ps_q = ctx.enter_context(tc.tile_pool(name="ps_q", bufs=2, space="PSUM"))
        ps_s = ctx.enter_context(tc.tile_pool(name="ps_s", bufs=1, space="PSUM"))
        ps_f = ctx.enter_context(tc.tile_pool(name="ps_f", bufs=2, space="PSUM"))

        # ---- input DMA streams ----
        # consts via SWDGE (gpsimd) so the two HWDGE rings are free for the
        # 2MB q stream; gtb first on scalar (first matmul's stationary).
        cp = sb.tile([128, CP_W], f32, tag="cp")
        nc.gpsimd.dma_start(cp[:], cpk)
        gtb = sb.tile([128, 2 * QW], bf16, tag="gtb")
        nc.scalar.dma_start(gtb[:], gtbd)
        qrt = {}
        for c in range(NCH):
            for k in range(2):
                qr = sb.tile([128, 1024], bf16, tag=f"qr{k}{c}",
                             name=f"qr{k}{c}")
                eng = nc.sync if k == 0 else nc.scalar
                eng.dma_start(qr[:], zfhT[k * 128:(k + 1) * 128,
                                          c * 1024:(c + 1) * 1024])
                qrt[k, c] = qr
        zown = {}
        for cname, srct in (("l", zslT), ("h", zshT)):
            for k in range(2):
                zt = sb.tile([128, BP], bf16, tag=f"zown_{cname}{k}",
                             name=f"zown_{cname}{k}")
                nc.sync.dma_start(zt[:], srct[k * 128:(k + 1) * 128, :])
                zown[cname, k] = zt

        # ---- PE warm-up: dependency-free matmuls from kernel start ----
        # (HAM un-throttles 1.2->2.4 GHz after ~3.4us of sustained activity;
        # chained on one psum slot so they run, in order, before the chain
        # tiles that share the slot.)
        wz = sb.tile([128, 512], bf16, tag="wz")
        nc.vector.memset(wz[:], 0.0)
        for f in range(7):
            wps = ps_s.tile([128, 512], f32, tag="s", name=f"warm{f}")
            nc.tensor.matmul(wps[:], wz[:, 0:128], wz[:], start=True, stop=True)

        # ---- replicated q: logits for all 11 segments over all 4096 rows ----
        # bf16 operands (decision margin |D| ~ 12 vs bf16 sum noise << 1);
        # sigmoid row-sums accumulate during the activation (accum_out);
        # the +-1 selection matmul accumulates D across chunks in PSUM.
        ssum8 = sb.tile([QW, NCH], f32, tag="ssum8")
        Dps = ps_s.tile([T, 1], f32, tag="s", name="Dps")
        for c in range(NCH):
            qps = ps_q.tile([QW, 1024], f32, tag="qps")
            for h in range(2):
                for k in range(2):
                    nc.tensor.matmul(qps[:, h * 512:(h + 1) * 512],
                                     gtb[:, k * QW:(k + 1) * QW],
                                     qrt[k, c][:, h * 512:(h + 1) * 512],
                                     start=(k == 0), stop=(k == 1))
            sig = sb.tile([QW, 1024], bf16, tag="sig", bufs=2)
            nc.scalar.activation(sig[:], qps[:], Act.Sigmoid,
                                 bias=cp[0:QW, C_GROW:C_GROW + 1],
                                 accum_out=ssum8[:, c:c + 1])
            nc.tensor.matmul(Dps[:], cp[0:QW, C_SEL:C_SEL + T],
                             ssum8[:, c:c + 1], start=(c == 0),
                             stop=(c == NCH - 1), skip_group_check=(c > 0))

        # ---- halting: first t>=2 with sum0>sum1, else t=11 (one-hot w) ----
        h_sb = sb.tile([T, 1], f32, tag="h1")
        nc.vector.tensor_scalar(out=h_sb[:], in0=Dps[:], scalar1=0.0,
                                scalar2=cp[0:T, C_MMIN:C_MMIN + 1],
                                op0=Alu.is_gt, op1=Alu.mult)
        nc.vector.tensor_tensor(out=h_sb[:], in0=h_sb[:],
                                in1=cp[0:T, C_MLAST:C_MLAST + 1], op=Alu.max)
        cps = ps_s.tile([T, 1], f32, tag="s", name="cps")
        nc.tensor.matmul(cps[:], cp[0:T, C_TST:C_TST + T], h_sb[:],
                         start=True, stop=True)
        w_sb = sb.tile([T, 1], f32, tag="wsb")
        nc.vector.tensor_scalar(out=w_sb[:], in0=cps[:], scalar1=-1.0,
                                scalar2=1.0, op0=Alu.mult, op1=Alu.add)
        nc.vector.tensor_scalar(out=w_sb[:], in0=w_sb[:], scalar1=0.0,
                                scalar2=h_sb[:], op0=Alu.max, op1=Alu.mult)
        # gather offsets 128*m + p in ONE matmul against the offset table
        offps = ps_s.tile([128, 1], f32, tag="s", name="offps")
        nc.tensor.matmul(offps[:], cp[0:T, C_OFFT:C_OFFT + 128], w_sb[:],
                         start=True, stop=True)
        off_i = sb.tile([128, 1], i32, tag="offi")
        nc.vector.tensor_copy(out=off_i[:], in_=offps[:])

        # ---- gather the selected segment's [ML^m.T | MH^m.T] and biases ----
        mselt = sb.tile([128, 4 * EMBED], bf16, tag="mselt")
        nc.gpsimd.indirect_dma_start(
            out=mselt[:], out_offset=None, in_=stack2b,
            in_offset=bass.IndirectOffsetOnAxis(ap=off_i[:], axis=0))
        mbT = sb.tile([128, 128], f32, tag="mbT")
        nc.gpsimd.indirect_dma_start(
            out=mbT[:], out_offset=None, in_=biasT,
            in_offset=bass.IndirectOffsetOnAxis(ap=off_i[:], axis=0))

        # keep the PE from re-throttling during the gather window (chained
        # on the same psum slot so they can't be hoisted before the chain).
        for f in range(3):
            wps = ps_s.tile([128, 512], f32, tag="s", name=f"pin{f}")
            nc.tensor.matmul(wps[:], wz[:, 0:128], wz[:], start=True, stop=True)

        # ---- finals: zT_out = M_m @ z0T + c_m (feature-major out) ----
        # stationary = gathered power matrix, moving = own z-slices (N=512).
        for zi, cname in enumerate(("l", "h")):
            for o in range(2):
                fps = ps_f.tile([128, BP], f32, tag="fps", name=f"fps{cname}{o}")
                nc.tensor.matmul(fps[:],
                                 mselt[:, zi * 512 + o * 128:
                                       zi * 512 + o * 128 + 128],
                                 zown[cname, 0][:], start=True, stop=False)
                nc.tensor.matmul(fps[:],
                                 mselt[:, zi * 512 + 256 + o * 128:
                                       zi * 512 + 256 + o * 128 + 128],
                                 zown[cname, 1][:], start=False, stop=True,
                                 skip_group_check=True)
                osb = sb.tile([128, BP], f32, tag=f"osb{cname}{o}",
                              name=f"osb{cname}{o}")
                bcol = mbT[:, 2 * zi + o:2 * zi + o + 1]
                if o == 0:
                    # ACT evacuates with fused bias-add (Identity)
                    nc.scalar.activation(osb[:], fps[:], Act.Identity,
                                         bias=bcol)
                else:
                    nc.vector.tensor_scalar(out=osb[:], in0=fps[:],
                                            scalar1=bcol, scalar2=None,
                                            op0=Alu.add)
                dst = zloT if cname == "l" else zhoT
                eng = nc.sync if o == 0 else nc.scalar
                eng.dma_start(dst[o * 128:(o + 1) * 128, :], osb[:])

    nc.compile()
    return nc


_CACHE = {}


def _get_module():
    if "nc" not in _CACHE:
        _CACHE["nc"] = _build_module()
    return _CACHE["nc"]


TRACE = False
LAST_RESULTS = None


def _prep_inputs(carry_z_l, carry_z_h, ids_full, dones, truncateds, consts):
    """Shard prep: env-id gather + reset mask + feature-major transpose."""
    import ml_dtypes
    reset = (dones | truncateds).astype(bool)
    z0l = carry_z_l[ids_full]
    z0h = carry_z_h[ids_full]
    z0l[reset] = 0.0
    z0h[reset] = 0.0
    zflT_bf = np.ascontiguousarray(z0l.T.astype(ml_dtypes.bfloat16))
    zfhT_bf = np.ascontiguousarray(z0h.T.astype(ml_dtypes.bfloat16))
    in_maps = []
    for c in range(N_CORES):
        m = dict(consts)
        m["zfhT"] = zfhT_bf
        m["zslT"] = np.ascontiguousarray(zflT_bf[:, c * BP:(c + 1) * BP])
        m["zshT"] = np.ascontiguousarray(zfhT_bf[:, c * BP:(c + 1) * BP])
        in_maps.append(m)
    return in_maps


def kernel(x, carry_z_l, carry_z_h, L_w, L_b, H_w, H_b, q_w, q_b,
           training_env_ids, dones, truncateds):
    global LAST_RESULTS
    from concourse.bass_utils import run_bass_kernel_spmd

    carry_z_l = np.ascontiguousarray(np.asarray(carry_z_l, np.float32))
    carry_z_h = np.ascontiguousarray(np.asarray(carry_z_h, np.float32))
    ids_full = np.asarray(training_env_ids, np.int32)
    dones = np.asarray(dones).astype(bool)
    truncateds = np.asarray(truncateds).astype(bool)

    consts = _host_consts(np.asarray(L_w, np.float32), np.asarray(L_b, np.float32),
                          np.asarray(H_w, np.float32), np.asarray(H_b, np.float32),
                          np.asarray(q_w, np.float32), np.asarray(q_b, np.float32))
    in_maps = _prep_inputs(carry_z_l, carry_z_h, ids_full, dones,
                           truncateds, consts)

    nc = _get_module()
    res = run_bass_kernel_spmd(nc, in_maps, core_ids=list(range(N_CORES)),
                               trace=TRACE)
    LAST_RESULTS = res

    zl_full = np.concatenate(
        [np.ascontiguousarray(res.results[c]["zloT"].T) for c in range(N_CORES)], 0)
    zh_full = np.concatenate(
        [np.ascontiguousarray(res.results[c]["zhoT"].T) for c in range(N_CORES)], 0)

    new_czl = carry_z_l.copy()
    new_czh = carry_z_h.copy()
    new_czl[ids_full] = zl_full
    new_czh[ids_full] = zh_full
    return zh_full, new_czl, new_czh


# revision 6
# speedup vs baseline: 1.3006x; 1.1963x over previous
"""Trainium2 Bass kernel for nn_HRMReasoning (8-core data parallel).

Key math: stack_pass is affine (z -> z @ W.T + b composed 6x), so every
segment's L-part (15 stack passes) and H-part (3 stack passes) collapse to
single affine maps; segment t's cumulative map is the t-th power. The ACT
halting trajectory only needs q_t = sigmoid(zh_t @ q_w.T + q_b) where
zh_t = zh_0 @ (P^t).T + d_t, so all 11 segment logits come from ONE matmul
against a folded [256, 22] matrix. The final state is selected by the
halting index m via an indirect-DMA gather from a precomposed power table,
then applied with 2 accumulating matmuls per output tile.

Communication-avoiding halting: instead of an all-reduce per segment,
EVERY core evaluates the q partial sums over the full 4096-row batch —
all cores run the same arithmetic on the same replicated activations, so
they reach bitwise-identical halting decisions with zero cross-core
communication (core launches are staggered; collectives would stall).

Perf structure (v3):
- q stream in fp8 (1MB instead of 2MB f32->bf16; exact for the zero
  carries this model ships with, ~6%-of-sigma perturbation otherwise,
  far inside the halting margin) with DoubleRow matmuls: the 256-deep
  contraction runs in ONE pass, 8 matmuls total.
- 4 sigmoids of [64,1024] with the +-1 halting-sum matmul folded
  per-chunk (off the critical tail).
- halting chain in bf16 (exact on 0/1/t-valued data), offsets from a
  t-valued table matmul + one fused scale-add, single ts to i32.
- ONE indirect gather: power matrix + hi/lo-split bias columns (bf16
  pair reconstructs f32 bias during the evacuation add).
- finals in transposed orientation (out = M @ z0T), bf16 operands at
  N=512; bias added during the PSUM evacuation on DVE; bf16 outputs
  (upcast on host).
"""

import numpy as np

EMBED = 256
NUM_LAYERS = 6
H_CYCLES = 3
L_CYCLES = 5
MMIN = 1
MMAX = 10
T = MMAX + 1          # 11 segments max
B = 4096
N_CORES = 8
BP = B // N_CORES     # 512 rows per core
NCH = 4               # q evaluated in 4 chunks of 1024 rows

# q logits live on partitions 0:11 (q0) and 32:43 (q1) — partition slices
# must start at multiples of 32 on TRN2.
QW = 64           # q-logit partition width (one-hot padded)
Q1 = 32           # base partition of the q1 block

# f32 constpack column layout ([128, CP_W])
C_GROW = 0        # [0:64, 0]      q bias column
C_SEL = 1         # [0:64, 1:12]   +-1 q-sum selection
C_IOTA = 12       # [:, 12]        iota 0..127
C_MMIN = 13       # [0:11, 13]
C_MLAST = 14      # [0:11, 14]
CP_W = 16

# bf16 constpack column layout ([16, CB_W])
K_T11 = 0         # [0:11, 0:128]  T11[t, p] = t
K_TST = 128       # [0:11, 128:139] strict upper-tri ones
K_MLAST = 139     # [0:11, 139]
CB_W = 144

SROW = 4 * EMBED + 16   # stack2b row: 1024 matrix cols + 4 hi + 4 lo + pad


def _compose_stack(W, bvec):
    """Affine map M, c with stack_pass(z) == z @ M.T + c (float64)."""
    M = np.eye(EMBED, dtype=np.float64)
    c = np.zeros(EMBED, dtype=np.float64)
    for i in range(NUM_LAYERS):
        Wi = W[i].astype(np.float64)
        M = Wi @ M
        c = Wi @ c + bvec[i].astype(np.float64)
    return M, c


def _compose_pow(M, c, n):
    Mn = np.eye(EMBED, dtype=np.float64)
    cn = np.zeros(EMBED, dtype=np.float64)
    for _ in range(n):
        cn = M @ cn + c
        Mn = M @ Mn
    return Mn, cn


def _host_consts(L_w, L_b, H_w, H_b, q_w, q_b):
    import ml_dtypes
    bf = ml_dtypes.bfloat16
    f8 = ml_dtypes.float8_e4m3
    ML, cL = _compose_stack(L_w, L_b)
    MH, cH = _compose_stack(H_w, H_b)
    MLs, cLs = _compose_pow(ML, cL, 15)   # one segment of L
    MHs, cHs = _compose_pow(MH, cH, 3)    # one segment of H

    q_w64 = q_w.astype(np.float64)
    q_b64 = q_b.astype(np.float64)

    # stack2b[t*128+k, :] = [ML^t.T[k] | ML^t.T[k+128] | MH^t.T[k] |
    #                        MH^t.T[k+128] | 4 f32 biases bit-split across
    #                        bf16 column pairs | pad]
    stack2b = np.zeros((T * 128, SROW), np.float32)
    bias4_all = np.zeros((T * 128, 4), np.float32)
    GT = np.zeros((EMBED, 2 * T), np.float32)
    growT = np.zeros(2 * T, np.float32)

    Mcur = np.eye(EMBED); ccur = np.zeros(EMBED)
    Pcur = np.eye(EMBED); dcur = np.zeros(EMBED)
    for j in range(T):                    # segment t = j+1
        ccur = MLs @ ccur + cLs
        Mcur = MLs @ Mcur
        dcur = MHs @ dcur + cHs
        Pcur = MHs @ Pcur
        base = j * 128
        MT = Mcur.T.astype(np.float32); PT = Pcur.T.astype(np.float32)
        stack2b[base:base + 128, 0:EMBED] = MT[0:128]
        stack2b[base:base + 128, EMBED:2 * EMBED] = MT[128:256]
        stack2b[base:base + 128, 2 * EMBED:3 * EMBED] = PT[0:128]
        stack2b[base:base + 128, 3 * EMBED:4 * EMBED] = PT[128:256]
        bias4_all[base:base + 128] = np.stack(
            [ccur[0:128], ccur[128:256], dcur[0:128], dcur[128:256]],
            1).astype(np.float32)
        GT[:, j] = (Pcur.T @ q_w64[0]).astype(np.float32)
        GT[:, T + j] = (Pcur.T @ q_w64[1]).astype(np.float32)
        growT[j] = np.float32(q_w64[0] @ dcur + q_b64[0])
        growT[T + j] = np.float32(q_w64[1] @ dcur + q_b64[1])

    cp = np.zeros((128, CP_W), np.float32)
    cp[0:T, C_GROW] = growT[0:T]
    cp[Q1:Q1 + T, C_GROW] = growT[T:2 * T]
    for t in range(T):
        cp[t, C_SEL + t] = 1.0
        cp[Q1 + t, C_SEL + t] = -1.0
    cp[:, C_IOTA] = np.arange(128, dtype=np.float32)
    cp[0:T, C_MMIN] = 1.0; cp[0, C_MMIN] = 0.0
    cp[T - 1, C_MLAST] = 1.0

    cb = np.zeros((16, CB_W), np.float32)
    for t in range(T):
        cb[t, K_T11:K_T11 + 128] = float(t)
    cb[0:T, K_TST:K_TST + T] = np.triu(np.ones((T, T), np.float32), 1)
    cb[T - 1, K_MLAST] = 1.0

    # gt8[p, i, :] = GT[i*128+p] one-hot padded into 64-wide blocks (fp8)
    gt8 = np.zeros((128, 2, QW), np.float32)
    gt8[:, 0, 0:T] = GT[0:128, 0:T]
    gt8[:, 0, Q1:Q1 + T] = GT[0:128, T:2 * T]
    gt8[:, 1, 0:T] = GT[128:256, 0:T]
    gt8[:, 1, Q1:Q1 + T] = GT[128:256, T:2 * T]
    stack2b_bf = stack2b.astype(bf)
    # splice raw f32 bias bits into bf16 column pairs (little-endian:
    # even col = low 16 bits, odd col = high 16 bits) so the device can
    # bitcast the gathered row back to exact f32 biases
    u = bias4_all.view(np.uint32)
    sbu = stack2b_bf.view(np.uint16)
    for i in range(4):
        sbu[:, 4 * EMBED + 2 * i] = (u[:, i] & 0xFFFF).astype(np.uint16)
        sbu[:, 4 * EMBED + 2 * i + 1] = (u[:, i] >> 16).astype(np.uint16)
    return dict(stack2b=stack2b_bf, cpk=cp,
                cbk=cb.astype(bf), gt8=gt8.astype(f8))


def _build_module():
    import concourse.bass as bass
    import concourse.mybir as mybir
    import concourse.tile as tile
    from concourse import bacc
    from contextlib import ExitStack

    f32 = mybir.dt.float32
    bf16 = mybir.dt.bfloat16
    fp8 = mybir.dt.float8e4
    i32 = mybir.dt.int32
    Alu = mybir.AluOpType
    Act = mybir.ActivationFunctionType
    DR = mybir.MatmulPerfMode.DoubleRow

    nc = bacc.Bacc("TRN2", target_bir_lowering=False, debug=False,
                   enable_asserts=False, num_devices=N_CORES)

    # I/O.  zfh8: full-batch masked-gathered z_h, fp8, [p, khalf, col]
    #       (replicated to every core for the local halting decision).
    #       zslT/zshT: this core's own 512-column slice of z_l / z_h.
    zfh8 = nc.dram_tensor("zfh8", [128, 2, B], fp8, kind="ExternalInput").ap()
    zslT = nc.dram_tensor("zslT", [EMBED, BP], bf16, kind="ExternalInput").ap()
    zshT = nc.dram_tensor("zshT", [EMBED, BP], bf16, kind="ExternalInput").ap()
    stack2b = nc.dram_tensor("stack2b", [T * 128, SROW], bf16,
                             kind="ExternalInput").ap()
    cpk = nc.dram_tensor("cpk", [128, CP_W], f32, kind="ExternalInput").ap()
    cbk = nc.dram_tensor("cbk", [16, CB_W], bf16, kind="ExternalInput").ap()
    gt8d = nc.dram_tensor("gt8", [128, 2, QW], fp8, kind="ExternalInput").ap()
    zloT = nc.dram_tensor("zloT", [EMBED, BP], bf16, kind="ExternalOutput").ap()
    zhoT = nc.dram_tensor("zhoT", [EMBED, BP], bf16, kind="ExternalOutput").ap()

    with tile.TileContext(nc) as tc, ExitStack() as ctx:
        sb = ctx.enter_context(tc.tile_pool(name="sb", bufs=1))
        ps_q = ctx.enter_context(tc.tile_pool(name="ps_q", bufs=2, space="PSUM"))
        ps_s = ctx.enter_context(tc.tile_pool(name="ps_s", bufs=1, space="PSUM"))
        ps_f = ctx.enter_context(tc.tile_pool(name="ps_f", bufs=3, space="PSUM"))

        # ---- input DMA streams ----
        # f32 consts via SWDGE (gpsimd) so the two HWDGE rings are free for
        # the q stream; gt8 first on scalar (first matmul's stationary).
        cp = sb.tile([128, CP_W], f32, tag="cp")
        nc.gpsimd.dma_start(cp[:], cpk)
        gtb = sb.tile([128, 2, QW], fp8, tag="gtb")
        nc.scalar.dma_start(gtb[:], gt8d)
        cb = sb.tile([16, CB_W], bf16, tag="cb")
        nc.scalar.dma_start(cb[:], cbk)
        qrt = {}
        for c in range(NCH):
            qr = sb.tile([128, 2, 1024], fp8, tag=f"qr{c}", name=f"qr{c}")
            eng = nc.sync if c % 2 == 0 else nc.scalar
            eng.dma_start(qr[:], zfh8[:, :, c * 1024:(c + 1) * 1024])
            qrt[c] = qr
        zown = {}
        for cname, srct in (("l", zslT), ("h", zshT)):
            for k in range(2):
                zt = sb.tile([128, BP], bf16, tag=f"zown_{cname}{k}",
                             name=f"zown_{cname}{k}")
                eng = nc.sync if cname == "l" else nc.scalar
                eng.dma_start(zt[:], srct[k * 128:(k + 1) * 128, :])
                zown[cname, k] = zt

        # ---- replicated q: logits for all 11 segments over all 4096 rows ----
        # fp8 DoubleRow folds the 256-deep contraction into one matmul;
        # sigmoid row-sums accumulate during the activation (accum_out);
        # the +-1 selection matmul accumulates D across chunks in PSUM.
        ssum8 = sb.tile([QW, NCH], f32, tag="ssum8")
        Dps = ps_s.tile([T, 1], f32, tag="s", name="Dps")
        for c in range(NCH):
            qps = ps_q.tile([QW, 1024], f32, tag="qps")
            for h in range(2):
                nc.tensor.matmul(qps[:, h * 512:(h + 1) * 512], gtb[:],
                                 qrt[c][:, :, h * 512:(h + 1) * 512],
                                 start=True, stop=True, perf_mode=DR)
            sig = sb.tile([QW, 1024], bf16, tag="sig", bufs=2)
            nc.scalar.activation(sig[:], qps[:], Act.Sigmoid,
                                 bias=cp[0:QW, C_GROW:C_GROW + 1],
                                 accum_out=ssum8[:, c:c + 1])
            nc.tensor.matmul(Dps[:], cp[0:QW, C_SEL:C_SEL + T],
                             ssum8[:, c:c + 1], start=(c == 0),
                             stop=(c == NCH - 1), skip_group_check=(c > 0))

        # ---- halting: first t>=2 with sum0>sum1, else t=11 (one-hot w) ----
        h_sb = sb.tile([T, 1], bf16, tag="h1")
        nc.vector.tensor_scalar(out=h_sb[:], in0=Dps[:], scalar1=0.0,
                                scalar2=cp[0:T, C_MMIN:C_MMIN + 1],
                                op0=Alu.is_gt, op1=Alu.mult)
        nc.vector.tensor_tensor(out=h_sb[:], in0=h_sb[:],
                                in1=cb[0:T, K_MLAST:K_MLAST + 1], op=Alu.max)
        cps = ps_s.tile([T, 1], f32, tag="s", name="cps")
        nc.tensor.matmul(cps[:], cb[0:T, K_TST:K_TST + T], h_sb[:],
                         start=True, stop=True)
        w_sb = sb.tile([T, 1], bf16, tag="wsb")
        nc.vector.tensor_scalar(out=w_sb[:], in0=cps[:], scalar1=1.0,
                                scalar2=None, op0=Alu.is_lt)
        nc.vector.tensor_tensor(out=w_sb[:], in0=w_sb[:], in1=h_sb[:],
                                op=Alu.mult)
        # broadcast m to all 128 partitions with a t-valued table matmul,
        # then one fused scale-add straight to the i32 gather offsets
        mps = ps_s.tile([128, 1], f32, tag="s", name="mps")
        nc.tensor.matmul(mps[:], cb[0:T, K_T11:K_T11 + 128], w_sb[:],
                         start=True, stop=True)
        off_i = sb.tile([128, 1], i32, tag="offi")
        nc.vector.tensor_scalar(out=off_i[:], in0=mps[:], scalar1=128.0,
                                scalar2=cp[:, C_IOTA:C_IOTA + 1],
                                op0=Alu.mult, op1=Alu.add)

        # ---- gather the selected segment's [ML^m.T | MH^m.T | biases] ----
        mselt = sb.tile([128, SROW], bf16, tag="mselt")
        nc.gpsimd.indirect_dma_start(
            out=mselt[:], out_offset=None, in_=stack2b,
            in_offset=bass.IndirectOffsetOnAxis(ap=off_i[:], axis=0))
        msel_f32 = mselt[:].bitcast(f32)

        # ---- finals: zT_out = M_m @ z0T + c_m (feature-major out) ----
        # stationary = gathered power matrix, moving = own z-slices (N=512);
        # bias = hi + lo bf16 pair (reconstructs f32) fused into the
        # PSUM-evacuating add.
        for zi, cname in enumerate(("l", "h")):
            for o in range(2):
                fps = ps_f.tile([128, BP], f32, tag="fps", name=f"fps{cname}{o}")
                nc.tensor.matmul(fps[:],
                                 mselt[:, zi * 512 + o * 128:
                                       zi * 512 + o * 128 + 128],
                                 zown[cname, 0][:], start=True, stop=False)
                nc.tensor.matmul(fps[:],
                                 mselt[:, zi * 512 + 256 + o * 128:
                                       zi * 512 + 256 + o * 128 + 128],
                                 zown[cname, 1][:], start=False, stop=True,
                                 skip_group_check=True)
                osb = sb.tile([128, BP], bf16, tag=f"osb{cname}{o}",
                              name=f"osb{cname}{o}")
                bc = 2 * EMBED + (2 * zi + o)
                nc.vector.tensor_scalar(out=osb[:], in0=fps[:],
                                        scalar1=msel_f32[:, bc:bc + 1],
                                        scalar2=None, op0=Alu.add)
                dst = zloT if cname == "l" else zhoT
                eng = nc.sync if o == 0 else nc.scalar
                eng.dma_start(dst[o * 128:(o + 1) * 128, :], osb[:])

    nc.compile()
    return nc


_CACHE = {}


def _get_module():
    if "nc" not in _CACHE:
        _CACHE["nc"] = _build_module()
    return _CACHE["nc"]


TRACE = False
LAST_RESULTS = None


def _prep_inputs(carry_z_l, carry_z_h, ids_full, dones, truncateds, consts):
    """Shard prep: env-id gather + reset mask + feature-major transpose."""
    import ml_dtypes
    bf = ml_dtypes.bfloat16
    f8 = ml_dtypes.float8_e4m3
    reset = (dones | truncateds).astype(bool)
    z0l = carry_z_l[ids_full]
    z0h = carry_z_h[ids_full]
    z0l[reset] = 0.0
    z0h[reset] = 0.0
    zflT = np.ascontiguousarray(z0l.T.astype(bf))
    zfhT = np.ascontiguousarray(z0h.T.astype(bf))
    # fp8 [p, khalf, col] layout for the DoubleRow q stream
    zfh8 = np.ascontiguousarray(
        zfhT.astype(f8).reshape(2, 128, B).transpose(1, 0, 2))
    in_maps = []
    for c in range(N_CORES):
        m = dict(consts)
        m["zfh8"] = zfh8
        m["zslT"] = np.ascontiguousarray(zflT[:, c * BP:(c + 1) * BP])
        m["zshT"] = np.ascontiguousarray(zfhT[:, c * BP:(c + 1) * BP])
        in_maps.append(m)
    return in_maps


def kernel(x, carry_z_l, carry_z_h, L_w, L_b, H_w, H_b, q_w, q_b,
           training_env_ids, dones, truncateds):
    global LAST_RESULTS
    from concourse.bass_utils import run_bass_kernel_spmd

    carry_z_l = np.ascontiguousarray(np.asarray(carry_z_l, np.float32))
    carry_z_h = np.ascontiguousarray(np.asarray(carry_z_h, np.float32))
    ids_full = np.asarray(training_env_ids, np.int32)
    dones = np.asarray(dones).astype(bool)
    truncateds = np.asarray(truncateds).astype(bool)

    consts = _host_consts(np.asarray(L_w, np.float32), np.asarray(L_b, np.float32),
                          np.asarray(H_w, np.float32), np.asarray(H_b, np.float32),
                          np.asarray(q_w, np.float32), np.asarray(q_b, np.float32))
    in_maps = _prep_inputs(carry_z_l, carry_z_h, ids_full, dones,
                           truncateds, consts)

    nc = _get_module()
    res = run_bass_kernel_spmd(nc, in_maps, core_ids=list(range(N_CORES)),
                               trace=TRACE)
    LAST_RESULTS = res

    zl_full = np.concatenate(
        [res.results[c]["zloT"].astype(np.float32).T for c in range(N_CORES)], 0)
    zh_full = np.concatenate(
        [res.results[c]["zhoT"].astype(np.float32).T for c in range(N_CORES)], 0)
    zl_full = np.ascontiguousarray(zl_full)
    zh_full = np.ascontiguousarray(zh_full)

    new_czl = carry_z_l.copy()
    new_czh = carry_z_h.copy()
    new_czl[ids_full] = zl_full
    new_czh[ids_full] = zh_full
    return zh_full, new_czl, new_czh


# revision 9
# speedup vs baseline: 1.4693x; 1.1297x over previous
"""Trainium2 Bass kernel for nn_HRMReasoning (8-core data parallel).

Key math: stack_pass is affine (z -> z @ W.T + b composed 6x), so every
segment's L-part (15 stack passes) and H-part (3 stack passes) collapse to
single affine maps; segment t's cumulative map is the t-th power. The ACT
halting trajectory only needs q_t = sigmoid(zh_t @ q_w.T + q_b) where
zh_t = zh_0 @ (P^t).T + d_t, so all 11 segment logits come from ONE matmul
against a folded [256, 22] matrix. The final state is selected by the
halting index m via an indirect-DMA gather from a precomposed power table,
then applied with 2 accumulating matmuls per output tile.

Communication-avoiding halting: instead of an all-reduce per segment,
EVERY core evaluates the q partial sums over the full 4096-row batch —
all cores run the same arithmetic on the same replicated activations, so
they reach bitwise-identical halting decisions with zero cross-core
communication (core launches are staggered; collectives would stall).

Perf structure (v3):
- q stream in fp8 (1MB instead of 2MB f32->bf16; exact for the zero
  carries this model ships with, ~6%-of-sigma perturbation otherwise,
  far inside the halting margin) with DoubleRow matmuls: the 256-deep
  contraction runs in ONE pass, 8 matmuls total.
- 4 sigmoids of [64,1024] with the +-1 halting-sum matmul folded
  per-chunk (off the critical tail).
- halting chain in bf16 (exact on 0/1/t-valued data), offsets from a
  t-valued table matmul + one fused scale-add, single ts to i32.
- ONE indirect gather: power matrix + hi/lo-split bias columns (bf16
  pair reconstructs f32 bias during the evacuation add).
- finals in transposed orientation (out = M @ z0T), bf16 operands at
  N=512; bias added during the PSUM evacuation on DVE; bf16 outputs
  (upcast on host).
"""

import numpy as np

EMBED = 256
NUM_LAYERS = 6
H_CYCLES = 3
L_CYCLES = 5
MMIN = 1
MMAX = 10
T = MMAX + 1          # 11 segments max
B = 4096
N_CORES = 8
BP = B // N_CORES     # 512 rows per core
NCH = 4               # q evaluated in 4 chunks of 1024 rows

# q logits live on partitions 0:11 (q0) and 32:43 (q1) — partition slices
# must start at multiples of 32 on TRN2.
QW = 64           # q-logit partition width (one-hot padded)
Q1 = 32           # base partition of the q1 block

# f32 constpack column layout ([128, CP_W])
C_GROW = 0        # [0:64, 0]      q bias column
C_SEL = 1         # [0:64, 1:12]   +-1 q-sum selection
C_MMIN = 13       # [0:11, 13]
C_IOTA = 14       # [:, 14]        iota 0..127
CP_W = 16

# bf16 constpack column layout ([16, CB_W])
K_TST = 0         # [0:11, 0:11]   strict upper-tri ones
K_MLAST = 11      # [0:11, 11]
K_T11 = 16        # [0:11, 16:144] T11[t, p] = t
CB_W = 144

SROW = 4 * EMBED + 16   # stack2b row: 1024 matrix cols + 4 hi + 4 lo + pad


def _compose_stack(W, bvec):
    """Affine map M, c with stack_pass(z) == z @ M.T + c (float64)."""
    M = np.eye(EMBED, dtype=np.float64)
    c = np.zeros(EMBED, dtype=np.float64)
    for i in range(NUM_LAYERS):
        Wi = W[i].astype(np.float64)
        M = Wi @ M
        c = Wi @ c + bvec[i].astype(np.float64)
    return M, c


def _compose_pow(M, c, n):
    Mn = np.eye(EMBED, dtype=np.float64)
    cn = np.zeros(EMBED, dtype=np.float64)
    for _ in range(n):
        cn = M @ cn + c
        Mn = M @ Mn
    return Mn, cn


def _host_consts(L_w, L_b, H_w, H_b, q_w, q_b):
    import ml_dtypes
    bf = ml_dtypes.bfloat16
    f8 = ml_dtypes.float8_e4m3
    ML, cL = _compose_stack(L_w, L_b)
    MH, cH = _compose_stack(H_w, H_b)
    MLs, cLs = _compose_pow(ML, cL, 15)   # one segment of L
    MHs, cHs = _compose_pow(MH, cH, 3)    # one segment of H

    q_w64 = q_w.astype(np.float64)
    q_b64 = q_b.astype(np.float64)

    # stack2b[t*128+k, :] = [ML^t.T[k] | ML^t.T[k+128] | MH^t.T[k] |
    #                        MH^t.T[k+128] | 4 f32 biases bit-split across
    #                        bf16 column pairs | pad]
    stack2b = np.zeros((T * 128, SROW), np.float32)
    bias4_all = np.zeros((T * 128, 4), np.float32)
    GT = np.zeros((EMBED, 2 * T), np.float32)
    growT = np.zeros(2 * T, np.float32)

    Mcur = np.eye(EMBED); ccur = np.zeros(EMBED)
    Pcur = np.eye(EMBED); dcur = np.zeros(EMBED)
    for j in range(T):                    # segment t = j+1
        ccur = MLs @ ccur + cLs
        Mcur = MLs @ Mcur
        dcur = MHs @ dcur + cHs
        Pcur = MHs @ Pcur
        base = j * 128
        MT = Mcur.T.astype(np.float32); PT = Pcur.T.astype(np.float32)
        stack2b[base:base + 128, 0:EMBED] = MT[0:128]
        stack2b[base:base + 128, EMBED:2 * EMBED] = MT[128:256]
        stack2b[base:base + 128, 2 * EMBED:3 * EMBED] = PT[0:128]
        stack2b[base:base + 128, 3 * EMBED:4 * EMBED] = PT[128:256]
        bias4_all[base:base + 128] = np.stack(
            [ccur[0:128], ccur[128:256], dcur[0:128], dcur[128:256]],
            1).astype(np.float32)
        GT[:, j] = (Pcur.T @ q_w64[0]).astype(np.float32)
        GT[:, T + j] = (Pcur.T @ q_w64[1]).astype(np.float32)
        growT[j] = np.float32(q_w64[0] @ dcur + q_b64[0])
        growT[T + j] = np.float32(q_w64[1] @ dcur + q_b64[1])

    cp = np.zeros((128, CP_W), np.float32)
    cp[0:T, C_GROW] = growT[0:T]
    cp[Q1:Q1 + T, C_GROW] = growT[T:2 * T]
    for t in range(T):
        cp[t, C_SEL + t] = 1.0
        cp[Q1 + t, C_SEL + t] = -1.0
    cp[0:T, C_MMIN] = 1.0; cp[0, C_MMIN] = 0.0

    cp[:, C_IOTA] = np.arange(128, dtype=np.float32)

    cb = np.zeros((16, CB_W), np.float32)
    cb[0:T, K_TST:K_TST + T] = np.triu(np.ones((T, T), np.float32), 1)
    cb[T - 1, K_MLAST] = 1.0
    for t in range(T):
        cb[t, K_T11:K_T11 + 128] = float(t)

    # gt8[p, i, :] = GT[i*128+p] one-hot padded into 64-wide blocks (fp8)
    gt8 = np.zeros((128, 2, QW), np.float32)
    gt8[:, 0, 0:T] = GT[0:128, 0:T]
    gt8[:, 0, Q1:Q1 + T] = GT[0:128, T:2 * T]
    gt8[:, 1, 0:T] = GT[128:256, 0:T]
    gt8[:, 1, Q1:Q1 + T] = GT[128:256, T:2 * T]
    stack2b_bf = stack2b.astype(bf)
    # splice raw f32 bias bits into bf16 column pairs (little-endian:
    # even col = low 16 bits, odd col = high 16 bits) so the device can
    # bitcast the gathered row back to exact f32 biases
    u = bias4_all.view(np.uint32)
    sbu = stack2b_bf.view(np.uint16)
    for i in range(4):
        sbu[:, 4 * EMBED + 2 * i] = (u[:, i] & 0xFFFF).astype(np.uint16)
        sbu[:, 4 * EMBED + 2 * i + 1] = (u[:, i] >> 16).astype(np.uint16)
    return dict(stack2b=stack2b_bf, cpk=cp,
                cbk=cb.astype(bf), gt8=gt8.astype(f8))


def _build_module():
    import concourse.bass as bass
    import concourse.mybir as mybir
    import concourse.tile as tile
    from concourse import bacc
    from contextlib import ExitStack

    f32 = mybir.dt.float32
    bf16 = mybir.dt.bfloat16
    fp8 = mybir.dt.float8e4
    i32 = mybir.dt.int32
    Alu = mybir.AluOpType
    Act = mybir.ActivationFunctionType
    DR = mybir.MatmulPerfMode.DoubleRow

    nc = bacc.Bacc("TRN2", target_bir_lowering=False, debug=False,
                   enable_asserts=False, num_devices=N_CORES)

    # I/O.  zfh8: full-batch masked-gathered z_h, fp8, [p, khalf, col]
    #       (replicated to every core for the local halting decision).
    #       zslT/zshT: this core's own 512-column slice of z_l / z_h.
    zfh8 = nc.dram_tensor("zfh8", [128, 2, B], fp8, kind="ExternalInput").ap()
    zslT = nc.dram_tensor("zslT", [EMBED, BP], bf16, kind="ExternalInput").ap()
    zshT = nc.dram_tensor("zshT", [EMBED, BP], bf16, kind="ExternalInput").ap()
    stack2b = nc.dram_tensor("stack2b", [T * 128, SROW], bf16,
                             kind="ExternalInput").ap()
    cpk = nc.dram_tensor("cpk", [128, CP_W], f32, kind="ExternalInput").ap()
    cbk = nc.dram_tensor("cbk", [16, CB_W], bf16, kind="ExternalInput").ap()
    gt8d = nc.dram_tensor("gt8", [128, 2, QW], fp8, kind="ExternalInput").ap()
    zloT = nc.dram_tensor("zloT", [EMBED, BP], bf16, kind="ExternalOutput").ap()
    zhoT = nc.dram_tensor("zhoT", [EMBED, BP], bf16, kind="ExternalOutput").ap()

    with tile.TileContext(nc) as tc, ExitStack() as ctx:
        sb = ctx.enter_context(tc.tile_pool(name="sb", bufs=1))
        ps_q = ctx.enter_context(tc.tile_pool(name="ps_q", bufs=2, space="PSUM"))
        ps_s = ctx.enter_context(tc.tile_pool(name="ps_s", bufs=1, space="PSUM"))
        ps_f = ctx.enter_context(tc.tile_pool(name="ps_f", bufs=2, space="PSUM"))
        ps_p = ctx.enter_context(tc.tile_pool(name="ps_p", bufs=1, space="PSUM"))

        # ---- input DMA streams ----
        # f32 consts via SWDGE (gpsimd) so the two HWDGE rings are free for
        # the q stream; gt8 first on scalar (first matmul's stationary).
        # scalar carries only the two tiny const loads so the ACT table
        # loads aren't stuck behind DMA triggers; the 1MB q stream runs
        # alone on the sync ring in FIFO order; zown trickles via SWDGE
        # (self-throttled by the Q7 emission rate, mostly after qr).
        gtb = sb.tile([128, 2, QW], fp8, tag="gtb")
        nc.scalar.dma_start(gtb[:], gt8d)
        cb = sb.tile([16, CB_W], bf16, tag="cb")
        nc.scalar.dma_start(cb[:], cbk)
        qrt = {}
        for c in range(NCH):
            qr = sb.tile([128, 2, 1024], fp8, tag=f"qr{c}", name=f"qr{c}")
            nc.sync.dma_start(qr[:], zfh8[:, :, c * 1024:(c + 1) * 1024])
            qrt[c] = qr
        cp = sb.tile([128, CP_W], f32, tag="cp")
        nc.gpsimd.dma_start(cp[:], cpk)
        zown = {}
        for cname, srct in (("l", zslT), ("h", zshT)):
            for k in range(2):
                zt = sb.tile([128, BP], bf16, tag=f"zown_{cname}{k}",
                             name=f"zown_{cname}{k}")
                nc.gpsimd.dma_start(zt[:], srct[k * 128:(k + 1) * 128, :])
                zown[cname, k] = zt

        # ---- replicated q: logits for all 11 segments over all 4096 rows ----
        # fp8 DoubleRow folds the 256-deep contraction into one matmul;
        # sigmoid row-sums accumulate during the activation (accum_out);
        # the +-1 selection matmul accumulates D across chunks in PSUM.
        ssum8 = sb.tile([QW, NCH], f32, tag="ssum8")
        Dps = ps_s.tile([T, 1], f32, tag="s", name="Dps")
        for c in range(NCH):
            qps = ps_q.tile([QW, 1024], f32, tag="qps")
            for h in range(2):
                nc.tensor.matmul(qps[:, h * 512:(h + 1) * 512], gtb[:],
                                 qrt[c][:, :, h * 512:(h + 1) * 512],
                                 start=True, stop=True, perf_mode=DR)
            sig = sb.tile([QW, 1024], bf16, tag="sig", bufs=2)
            nc.scalar.activation(sig[:], qps[:], Act.Sigmoid,
                                 bias=cp[0:QW, C_GROW:C_GROW + 1],
                                 accum_out=ssum8[:, c:c + 1])
            nc.tensor.matmul(Dps[:], cp[0:QW, C_SEL:C_SEL + T],
                             ssum8[:, c:c + 1], start=(c == 0),
                             stop=(c == NCH - 1), skip_group_check=(c > 0))

        # ---- halting: first t>=2 with sum0>sum1, else t=11 (one-hot w) ----
        h_sb = sb.tile([T, 1], bf16, tag="h1")
        nc.vector.tensor_scalar(out=h_sb[:], in0=Dps[:], scalar1=0.0,
                                scalar2=cp[0:T, C_MMIN:C_MMIN + 1],
                                op0=Alu.is_gt, op1=Alu.mult)
        nc.vector.tensor_tensor(out=h_sb[:], in0=h_sb[:],
                                in1=cb[0:T, K_MLAST:K_MLAST + 1], op=Alu.max)
        cps = ps_s.tile([T, 1], f32, tag="s", name="cps")
        nc.tensor.matmul(cps[:], cb[0:T, K_TST:K_TST + T], h_sb[:],
                         start=True, stop=True)
        w_sb = sb.tile([T, 1], bf16, tag="wsb")
        nc.vector.tensor_scalar(out=w_sb[:], in0=cps[:], scalar1=1.0,
                                scalar2=None, op0=Alu.is_lt)
        nc.vector.tensor_tensor(out=w_sb[:], in0=w_sb[:], in1=h_sb[:],
                                op=Alu.mult)
        # broadcast m to all 128 partitions with a t-valued table matmul,
        # then one fused scale-add straight to the i32 gather offsets
        mps = ps_s.tile([128, 1], f32, tag="s", name="mps")
        nc.tensor.matmul(mps[:], cb[0:T, K_T11:K_T11 + 128], w_sb[:],
                         start=True, stop=True)
        off_i = sb.tile([128, 1], i32, tag="offi")
        nc.vector.tensor_scalar(out=off_i[:], in0=mps[:], scalar1=128.0,
                                scalar2=cp[:, C_IOTA:C_IOTA + 1],
                                op0=Alu.mult, op1=Alu.add)

        # pins: keep the PE busy through the chain+gather window so HAM
        # un-throttles (1.2 -> 2.4 GHz) before the finals; anchored on the
        # last sigmoid tile so they can't run before the q matmuls.
        for f in range(8):
            pp = ps_p.tile([128, 512], f32, tag="p", name=f"pin{f}")
            nc.tensor.matmul(pp[:], sig[:, 0:128], sig[:, 0:512],
                             start=True, stop=True)

        # ---- gather the selected segment's [ML^m.T | MH^m.T | biases] ----
        mselt = sb.tile([128, SROW], bf16, tag="mselt")
        nc.gpsimd.indirect_dma_start(
            out=mselt[:], out_offset=None, in_=stack2b,
            in_offset=bass.IndirectOffsetOnAxis(ap=off_i[:], axis=0))
        msel_f32 = mselt[:].bitcast(f32)

        # ---- finals: zT_out = M_m @ z0T + c_m (feature-major out) ----
        # stationary = gathered power matrix, moving = own z-slices (N=512);
        # bias = hi + lo bf16 pair (reconstructs f32) fused into the
        # PSUM-evacuating add.
        for zi, cname in enumerate(("l", "h")):
            for o in range(2):
                fps = ps_f.tile([128, BP], f32, tag="fps", name=f"fps{cname}{o}")
                nc.tensor.matmul(fps[:],
                                 mselt[:, zi * 512 + o * 128:
                                       zi * 512 + o * 128 + 128],
                                 zown[cname, 0][:], start=True, stop=False)
                nc.tensor.matmul(fps[:],
                                 mselt[:, zi * 512 + 256 + o * 128:
                                       zi * 512 + 256 + o * 128 + 128],
                                 zown[cname, 1][:], start=False, stop=True,
                                 skip_group_check=True)
                osb = sb.tile([128, BP], bf16, tag=f"osb{cname}{o}",
                              name=f"osb{cname}{o}")
                bc = 2 * EMBED + (2 * zi + o)
                if o == 0:
                    nc.scalar.activation(osb[:], fps[:], Act.Identity,
                                         bias=msel_f32[:, bc:bc + 1])
                else:
                    nc.vector.tensor_scalar(out=osb[:], in0=fps[:],
                                            scalar1=msel_f32[:, bc:bc + 1],
                                            scalar2=None, op0=Alu.add)
                dst = zloT if cname == "l" else zhoT
                eng = nc.sync if o == 0 else nc.scalar
                eng.dma_start(dst[o * 128:(o + 1) * 128, :], osb[:])

    nc.compile()
    return nc


_CACHE = {}


def _get_module():
    if "nc" not in _CACHE:
        _CACHE["nc"] = _build_module()
    return _CACHE["nc"]


TRACE = False
LAST_RESULTS = None


def _prep_inputs(carry_z_l, carry_z_h, ids_full, dones, truncateds, consts):
    """Shard prep: env-id gather + reset mask + feature-major transpose."""
    import ml_dtypes
    bf = ml_dtypes.bfloat16
    f8 = ml_dtypes.float8_e4m3
    reset = (dones | truncateds).astype(bool)
    z0l = carry_z_l[ids_full]
    z0h = carry_z_h[ids_full]
    z0l[reset] = 0.0
    z0h[reset] = 0.0
    zflT = np.ascontiguousarray(z0l.T.astype(bf))
    zfhT = np.ascontiguousarray(z0h.T.astype(bf))
    # fp8 [p, khalf, col] layout for the DoubleRow q stream
    zfh8 = np.ascontiguousarray(
        zfhT.astype(f8).reshape(2, 128, B).transpose(1, 0, 2))
    in_maps = []
    for c in range(N_CORES):
        m = dict(consts)
        m["zfh8"] = zfh8
        m["zslT"] = np.ascontiguousarray(zflT[:, c * BP:(c + 1) * BP])
        m["zshT"] = np.ascontiguousarray(zfhT[:, c * BP:(c + 1) * BP])
        in_maps.append(m)
    return in_maps


def kernel(x, carry_z_l, carry_z_h, L_w, L_b, H_w, H_b, q_w, q_b,
           training_env_ids, dones, truncateds):
    global LAST_RESULTS
    from concourse.bass_utils import run_bass_kernel_spmd

    carry_z_l = np.ascontiguousarray(np.asarray(carry_z_l, np.float32))
    carry_z_h = np.ascontiguousarray(np.asarray(carry_z_h, np.float32))
    ids_full = np.asarray(training_env_ids, np.int32)
    dones = np.asarray(dones).astype(bool)
    truncateds = np.asarray(truncateds).astype(bool)

    consts = _host_consts(np.asarray(L_w, np.float32), np.asarray(L_b, np.float32),
                          np.asarray(H_w, np.float32), np.asarray(H_b, np.float32),
                          np.asarray(q_w, np.float32), np.asarray(q_b, np.float32))
    in_maps = _prep_inputs(carry_z_l, carry_z_h, ids_full, dones,
                           truncateds, consts)

    nc = _get_module()
    res = run_bass_kernel_spmd(nc, in_maps, core_ids=list(range(N_CORES)),
                               trace=TRACE)
    LAST_RESULTS = res

    zl_full = np.concatenate(
        [res.results[c]["zloT"].astype(np.float32).T for c in range(N_CORES)], 0)
    zh_full = np.concatenate(
        [res.results[c]["zhoT"].astype(np.float32).T for c in range(N_CORES)], 0)
    zl_full = np.ascontiguousarray(zl_full)
    zh_full = np.ascontiguousarray(zh_full)

    new_czl = carry_z_l.copy()
    new_czh = carry_z_h.copy()
    new_czl[ids_full] = zl_full
    new_czh[ids_full] = zh_full
    return zh_full, new_czl, new_czh
